# revision 14
# baseline (speedup 1.0000x reference)
"""BidirectionalAttention Trainium2 Bass kernel — 8-core SPMD.

Decomposition (verified against the oracle in fp32, rel-err 2.9e-7):
  q path : 1x1 conv (matmul) -> grouped conv1d k=3 -> conv1d k=3
  attn   : E = exp(q^T k) without max-subtraction (attn absmax ~6.5);
           both softmaxes share one exp:
             attn_f + attn_b = E * (1/S0[n,m] + 1/S1[b,m])
             S0 = sum_b E  (batch softmax denom, axis=0)
             S1 = sum_n E  (row softmax denom, axis=1)
  fusion = value @ (attn_f+attn_b)^T scaled by gamma*mean(x_b), + x
  ConvTranspose2d(k=4,s=2,p=1) via the 4-subkernel parity decomposition.

Sharding: sequence-parallel over attention rows n (HW=4096 -> 512 rows/core =
8 image rows).  Per core E is stored transposed [m, (b, n_loc)] in SBUF
(bf16, 32 tiles of [128, 4, 512], one exp per m-tile):
  - S0 (sum over batch) is local elementwise over the 4 batch slices
  - S1 (sum over n) is a free-dim sum (DVE tensor_scalar accum_out), then
    two small AllReduces (split in half so the first half of the fusion
    matmuls can start while the second half of QK/exp still runs).
K and V^T shards are exchanged with small AllGathers (K per batch, early).
The ConvTranspose needs fusion rows h0-1..h0+8; instead of a halo exchange
each core emits an 18-row output slab with *partial* sums on the 2-row
boundaries and the host stitches slabs by adding the overlaps
(transposed-conv contributions are additive), keeping the device program
rank-independent.
"""

import numpy as np

B = 4
C = 256
H = 64
Wd = 64
HW = H * Wd            # 4096
CR = 32                # C // 8
NCORES = 8
NL = HW // NCORES      # 512 owned attention rows (n) per core
HL = H // NCORES       # 8 owned image rows per core
MT = HW // 128         # 32 m-tiles of 128
XW = NL + 4            # x slab width (n halo +-2 for the two k=3 convs)
Q2W = NL + 2           # q2 width (halo +-1 for conv2)
ROWW = 68              # fusion_pad row width: [0,1]=zero, 2..65 data, [66,67]=zero
OUTROWS = 2 * HL + 2   # 18 output rows per core (2-row overlaps, host-stitched)

_CACHE = {}


# ---------------------------------------------------------------------------
# device module
# ---------------------------------------------------------------------------
def build_module():
    from contextlib import ExitStack

    import concourse.bass as bass
    import concourse.mybir as mybir
    from concourse import bacc
    from concourse.tile import TileContext

    f32 = mybir.dt.float32
    bf16 = mybir.dt.bfloat16
    AF = mybir.ActivationFunctionType
    OP = mybir.AluOpType
    AX = mybir.AxisListType

    nc = bacc.Bacc(num_devices=NCORES)
    RG = [list(range(NCORES))]

    # ---- parameters (per-core) -------------------------------------------
    xs_p = nc.declare_dram_parameter("xs", [B, C, XW], bf16, isOutput=False)
    wqT_p = nc.declare_dram_parameter("wqT", [C, C], bf16, isOutput=False)
    wvT_p = nc.declare_dram_parameter("wvT", [C, C], bf16, isOutput=False)
    w1_p = nc.declare_dram_parameter("w1", [3, C, CR], bf16, isOutput=False)
    w2_p = nc.declare_dram_parameter("w2", [3, CR, 2 * CR], bf16, isOutput=False)
    wco_p = nc.declare_dram_parameter("wco", [4, 4, C, C // 2], bf16, isOutput=False)
    bq_p = nc.declare_dram_parameter("bq", [C, 1], f32, isOutput=False)
    b1_p = nc.declare_dram_parameter("b1", [CR, 1], f32, isOutput=False)
    b2_p = nc.declare_dram_parameter("b2p", [2 * CR, 1], f32, isOutput=False)
    bco_p = nc.declare_dram_parameter("bco", [C // 2, 1], f32, isOutput=False)
    bvb_p = nc.declare_dram_parameter("bvb", [128, C], bf16, isOutput=False)
    mask_p = nc.declare_dram_parameter("mask", [128, XW], bf16, isOutput=False)
    gamma_p = nc.declare_dram_parameter("gammas", [1, 1], f32, isOutput=False)
    out_p = nc.declare_dram_parameter(
        "out", [B, C // 2, OUTROWS, 2 * Wd], f32, isOutput=True
    )

    with TileContext(nc) as tc, ExitStack() as ctx:
        # ---- long-lived pools -------------------------------------------
        const = ctx.enter_context(tc.tile_pool(name="const", bufs=1))
        xpool = ctx.enter_context(tc.tile_pool(name="xp", bufs=1))
        qkv = ctx.enter_context(tc.tile_pool(name="qkv", bufs=1))
        epool = ctx.enter_context(tc.tile_pool(name="E", bufs=1))
        fpool = ctx.enter_context(tc.tile_pool(name="fp", bufs=1))
        dram = ctx.enter_context(tc.tile_pool(name="dram", bufs=1, space="DRAM"))

        # ---- DRAM bounce buffers ----------------------------------------
        k_in = [dram.tile([CR, NL], bf16, tag=f"k_in{b}", name=f"k_in{b}") for b in range(B)]
        k_out = [
            dram.tile([NCORES, CR, NL], bf16, tag=f"k_out{b}", name=f"k_out{b}")
            for b in range(B)
        ]
        v_in = dram.tile([B, NL, C], bf16, tag="v_in", name="v_in")
        v_out = dram.tile([NCORES, B, NL, C], bf16, tag="v_out", name="v_out")
        ar1_in = dram.tile([128, 64], f32, tag="ar1_in", name="ar1_in")
        ar1_out = dram.tile([128, 64], f32, tag="ar1_out", name="ar1_out")
        ar2_in = dram.tile([128, 72], f32, tag="ar2_in", name="ar2_in")
        ar2_out = dram.tile([128, 72], f32, tag="ar2_out", name="ar2_out")
        g_dram = dram.tile([1, B], f32, tag="g_dram", name="g_dram")

        # ---- constants into SBUF ----------------------------------------
        wq_sb = [const.tile([128, C], bf16, tag=f"wq{k}", name=f"wq{k}") for k in range(2)]
        wv_sb = [const.tile([128, C], bf16, tag=f"wv{k}", name=f"wv{k}") for k in range(2)]
        for k in range(2):
            nc.sync.dma_start(out=wq_sb[k], in_=wqT_p[k * 128 : (k + 1) * 128, :])
            nc.sync.dma_start(out=wv_sb[k], in_=wvT_p[k * 128 : (k + 1) * 128, :])
        w1_sb = [
            [const.tile([128, CR], bf16, tag=f"w1_{t}_{k}", name=f"w1_{t}_{k}") for k in range(2)]
            for t in range(3)
        ]
        for t in range(3):
            for k in range(2):
                nc.sync.dma_start(
                    out=w1_sb[t][k], in_=w1_p[t, k * 128 : (k + 1) * 128, :]
                )
        w2_sb = [const.tile([CR, 2 * CR], bf16, tag=f"w2_{t}", name=f"w2_{t}") for t in range(3)]
        for t in range(3):
            nc.sync.dma_start(out=w2_sb[t], in_=w2_p[t])
        bq_sb = [const.tile([128, 1], f32, tag=f"bq{k}", name=f"bq{k}") for k in range(2)]
        for k in range(2):
            nc.sync.dma_start(out=bq_sb[k], in_=bq_p[k * 128 : (k + 1) * 128, :])
        b1_sb = const.tile([CR, 1], f32, tag="b1", name="b1")
        nc.sync.dma_start(out=b1_sb, in_=b1_p[:, :])
        b2_sb = const.tile([2 * CR, 1], f32, tag="b2", name="b2")
        nc.sync.dma_start(out=b2_sb, in_=b2_p[:, :])
        bco_sb = const.tile([128, 1], f32, tag="bco", name="bco")
        nc.sync.dma_start(out=bco_sb, in_=bco_p[:, :])
        bvb_sb = const.tile([128, C], bf16, tag="bvb", name="bvb")
        nc.sync.dma_start(out=bvb_sb, in_=bvb_p[:, :])
        mask_sb = const.tile([128, XW], bf16, tag="mask", name="mask")
        nc.sync.dma_start(out=mask_sb, in_=mask_p[:, :])
        gm_sb = const.tile([1, 1], f32, tag="gm", name="gm")
        nc.sync.dma_start(out=gm_sb, in_=gamma_p[:, :])
        wco_sb = [
            [
                [const.tile([128, 128], bf16, tag=f"wco{ky}_{kx}_{k}", name=f"wco{ky}_{kx}_{k}") for k in range(2)]
                for kx in range(4)
            ]
            for ky in range(4)
        ]
        for ky in range(4):
            for kx in range(4):
                for k in range(2):
                    nc.sync.dma_start(
                        out=wco_sb[ky][kx][k],
                        in_=wco_p[ky, kx, k * 128 : (k + 1) * 128, :],
                    )

        # ---- x load (already bf16 + zero-padded halo on host) -----------
        x_sb = [
            [xpool.tile([128, XW], bf16, tag=f"x{b}_{k}", name=f"x{b}_{k}") for k in range(2)]
            for b in range(B)
        ]
        for b in range(B):
            for k in range(2):
                nc.sync.dma_start(
                    out=x_sb[b][k], in_=xs_p[b, k * 128 : (k + 1) * 128, :]
                )

        tc.strict_bb_all_engine_barrier()

        # s1p: S1 partials at col mt*4+b (cols 0..127), x partial sums at
        # cols 128 + b*2 + k.  AllReduced in two halves.
        s1p_sb = qkv.tile([128, 136], f32, tag="s1p", name="s1p")
        for b in range(B):
            for k in range(2):
                cc = 128 + b * 2 + k
                nc.vector.tensor_reduce(
                    out=s1p_sb[:, cc : cc + 1],
                    in_=x_sb[b][k][:, 2 : 2 + NL],
                    axis=AX.X,
                    op=OP.add,
                )

        Q_all = qkv.tile([128, NL], bf16, tag="Q", name="Q")
        K_all = qkv.tile([128, HW], bf16, tag="K", name="K")
        r1_sb = qkv.tile([128, 128], f32, tag="r1", name="r1")  # 1/S1, col mt*4+b
        g_bcast = qkv.tile([128, B], f32, tag="gbc", name="gbc")

        # =================================================================
        # phase A: q path (per batch); phase B: value path
        # =================================================================
        with (
            tc.tile_pool(name="qtmp", bufs=2) as qtmp,
            tc.tile_pool(name="qps", bufs=2, space="PSUM") as qps,
            tc.tile_pool(name="q2ps", bufs=1, space="PSUM") as q2ps,
            tc.tile_pool(name="q3ps", bufs=1, space="PSUM") as q3ps,
            tc.tile_pool(name="vps", bufs=1, space="PSUM") as vps,
        ):
            for b in range(B):
                # ---- q1 = wq @ x + bq, then edge-mask -------------------
                q1_sb = []
                for mtile in range(2):
                    ps = qps.tile([128, XW], f32, tag="q1ps", name="q1ps")
                    for k in range(2):
                        for lo, hi in ((0, 512), (512, XW)):
                            nc.tensor.matmul(
                                ps[:, lo:hi],
                                wq_sb[k][:, mtile * 128 : (mtile + 1) * 128],
                                x_sb[b][k][:, lo:hi],
                                start=(k == 0),
                                stop=(k == 1),
                            )
                    q1 = qtmp.tile([128, XW], bf16, tag=f"q1_{mtile}", name=f"q1_{mtile}")
                    nc.scalar.activation(
                        out=q1, in_=ps, func=AF.Identity, bias=bq_sb[mtile]
                    )
                    nc.vector.tensor_mul(q1, q1, mask_sb)
                    q1_sb.append(q1)

                # ---- q2 = groupedconv(q1) + b1, then edge-mask ----------
                ps2 = q2ps.tile([CR, Q2W], f32, tag="q2ps", name="q2ps")
                for t in range(3):
                    for k in range(2):
                        st = t == 0 and k == 0
                        sp = t == 2 and k == 1
                        for lo, hi in ((0, 512), (512, Q2W)):
                            nc.tensor.matmul(
                                ps2[:, lo:hi],
                                w1_sb[t][k],
                                q1_sb[k][:, lo + t : hi + t],
                                start=st,
                                stop=sp,
                            )
                q2 = qtmp.tile([CR, Q2W], bf16, tag="q2", name="q2")
                nc.scalar.activation(out=q2, in_=ps2, func=AF.Identity, bias=b1_sb)
                nc.vector.tensor_mul(q2, q2, mask_sb[:CR, 1 : 1 + Q2W])

                # ---- q3 = conv(q2) + b2 (rows 0..31 query, 32..63 key) --
                ps3 = q3ps.tile([2 * CR, NL], f32, tag="q3ps", name="q3ps")
                for t in range(3):
                    nc.tensor.matmul(
                        ps3,
                        w2_sb[t],
                        q2[:, t : t + NL],
                        start=(t == 0),
                        stop=(t == 2),
                    )
                q3 = qtmp.tile([2 * CR, NL], bf16, tag="q3", name="q3")
                nc.scalar.activation(out=q3, in_=ps3, func=AF.Identity, bias=b2_sb)
                nc.sync.dma_start(
                    out=Q_all[32 * b : 32 * b + 32, :], in_=q3[0:CR, :]
                )
                nc.sync.dma_start(out=k_in[b][:, :], in_=q3[CR : 2 * CR, :])
                # gather this batch's key shard early (overlaps the rest)
                nc.gpsimd.collective_compute(
                    "AllGather",
                    OP.bypass,
                    replica_groups=RG,
                    ins=[k_in[b][:, :]],
                    outs=[k_out[b][:, :, :]],
                )
                nc.sync.dma_start(
                    out=K_all[32 * b : 32 * b + 32, :].rearrange(
                        "c (g m) -> c g m", g=NCORES
                    ),
                    in_=k_out[b][:, :, :].rearrange("g c m -> c g m"),
                )

                # ---- value^T shard: [m, c] = x^T @ wv^T + bv ------------
                for ms in range(4):
                    psv = vps.tile([128, C], f32, tag="vps", name="vps")
                    for k in range(2):
                        nc.tensor.matmul(
                            psv,
                            x_sb[b][k][:, 2 + ms * 128 : 2 + (ms + 1) * 128],
                            wv_sb[k],
                            start=(k == 0),
                            stop=(k == 1),
                        )
                    vt = qtmp.tile([128, C], bf16, tag="vt", name="vt")
                    nc.vector.tensor_add(vt, psv, bvb_sb)
                    nc.sync.dma_start(
                        out=v_in[b, ms * 128 : (ms + 1) * 128, :], in_=vt
                    )

        nc.gpsimd.collective_compute(
            "AllGather",
            OP.bypass,
            replica_groups=RG,
            ins=[v_in[:, :, :]],
            outs=[v_out[:, :, :, :]],
        )

        # =================================================================
        # phase C: E = exp(K^T Q), one [128, 4x512] tile per m-tile.
        # S1 partials via DVE tensor_scalar accum; AllReduce in two halves.
        # =================================================================
        e_sb = [None] * MT
        with (
            tc.tile_pool(name="qk", bufs=2, space="PSUM") as qk,
            tc.tile_pool(name="sc", bufs=2) as sc,
        ):
            for mt in range(MT):
                ps4 = qk.tile([128, B, NL], f32, tag="e4ps", name="e4ps")
                for b in range(B):
                    nc.tensor.matmul(
                        ps4[:, b, :],
                        K_all[32 * b : 32 * b + 32, mt * 128 : (mt + 1) * 128],
                        Q_all[32 * b : 32 * b + 32, :],
                        start=True,
                        stop=True,
                        tile_position=(32 * b, 0),
                    )
                e4 = epool.tile([128, B, NL], bf16, tag=f"e{mt}", name=f"e{mt}")
                nc.scalar.activation(out=e4, in_=ps4, func=AF.Exp)
                e_sb[mt] = e4
                # S1 partials: free-dim accumulate on DVE (4x mode copy)
                for b in range(B):
                    scr = sc.tile([128, NL], bf16, tag="scr", name="scr")
                    col = mt * 4 + b
                    nc.vector.tensor_scalar(
                        out=scr,
                        in0=e4[:, b, :],
                        scalar1=1.0,
                        scalar2=None,
                        op0=OP.mult,
                        op1=OP.add,
                        accum_out=s1p_sb[:, col : col + 1],
                    )

                if mt == MT // 2 - 1:
                    nc.sync.dma_start(out=ar1_in[:, :], in_=s1p_sb[:, 0:64])
                    nc.gpsimd.collective_compute(
                        "AllReduce", OP.add, replica_groups=RG,
                        ins=[ar1_in[:, :]], outs=[ar1_out[:, :]],
                    )
                    a1o = qkv.tile([128, 64], f32, tag="a1o", name="a1o")
                    nc.sync.dma_start(out=a1o, in_=ar1_out[:, :])
                    nc.vector.reciprocal_approx_fast(out=r1_sb[:, 0:64], in_=a1o)

            # second AR half: S1 cols 64..128 plus the x sums
            nc.sync.dma_start(out=ar2_in[:, 0:64], in_=s1p_sb[:, 64:128])
            nc.sync.dma_start(out=ar2_in[:, 64:72], in_=s1p_sb[:, 128:136])
            nc.gpsimd.collective_compute(
                "AllReduce", OP.add, replica_groups=RG,
                ins=[ar2_in[:, :]], outs=[ar2_out[:, :]],
            )
            a2o = qkv.tile([128, 72], f32, tag="a2o", name="a2o")
            nc.sync.dma_start(out=a2o, in_=ar2_out[:, :])
            nc.vector.reciprocal_approx_fast(out=r1_sb[:, 64:128], in_=a2o[:, 0:64])

            # g_bcast[p, b] = gamma * mean(x[b]): partition-reduce on gpsimd,
            # tiny math on partition 0, broadcast via 0-stride DMA from DRAM.
            xps = sc.tile([1, 8], f32, tag="xps", name="xps")
            nc.gpsimd.tensor_reduce(
                out=xps, in_=a2o[:, 64:72], axis=AX.C, op=OP.add
            )
            xv = xps.rearrange("p (b k) -> p b k", b=B)
            g0 = sc.tile([1, B], f32, tag="g0", name="g0")
            nc.vector.tensor_add(g0, xv[:, :, 0], xv[:, :, 1])
            nc.vector.tensor_scalar(
                out=g0,
                in0=g0,
                scalar1=gm_sb,
                scalar2=float(1.0 / (C * HW)),
                op0=OP.mult,
                op1=OP.mult,
            )
            nc.sync.dma_start(out=g_dram[:, :], in_=g0)
            nc.sync.dma_start(
                out=g_bcast,
                in_=bass.AP(
                    tensor=g_dram.tensor,
                    offset=g_dram.offset,
                    ap=[[0, 128], [1, B]],
                ),
            )

        # =================================================================
        # phase D: R = 1/S0; A = E*(R + r1b) in place; fusion matmuls
        # =================================================================
        fp_sb = [
            [fpool.tile([128, 10, ROWW], bf16, tag=f"fpad{b}_{ch}", name=f"fpad{b}_{ch}") for ch in range(2)]
            for b in range(B)
        ]
        for b in range(B):
            for ch in range(2):
                nc.gpsimd.memset(fp_sb[b][ch], 0.0)

        with (
            tc.tile_pool(name="fus", bufs=1, space="PSUM") as fus,
            tc.tile_pool(name="vtp", bufs=4) as vtp,
            tc.tile_pool(name="sp2", bufs=2) as sp2,
        ):
            fusion_ps = [
                [fus.tile([128, NL], f32, tag=f"f{b}_{ch}", name=f"f{b}_{ch}") for ch in range(2)]
                for b in range(B)
            ]
            for mt in range(MT):
                e4 = e_sb[mt]
                # S0 = sum_b E on gpsimd (idle engine), recip+cast on DVE
                s01 = sp2.tile([128, NL], bf16, tag="s01", name="s01")
                nc.gpsimd.tensor_add(s01, e4[:, 0, :], e4[:, 1, :])
                s23 = sp2.tile([128, NL], bf16, tag="s23", name="s23")
                nc.gpsimd.tensor_add(s23, e4[:, 2, :], e4[:, 3, :])
                s0f = sp2.tile([128, NL], f32, tag="s0f", name="s0f")
                nc.gpsimd.tensor_add(s0f, s01, s23)
                rf = sp2.tile([128, NL], f32, tag="rf", name="rf")
                nc.vector.reciprocal_approx_fast(out=rf, in_=s0f)
                rb = sp2.tile([128, NL], bf16, tag="rb", name="rb")
                nc.vector.tensor_copy(rb, rf)
                # tmp4[:, b] = R + 1/S1[b]; A = tmp4 * E in one wide mul
                tmp4 = sp2.tile([128, B, NL], bf16, tag="tmp4", name="tmp4")
                for b in range(B):
                    col = mt * 4 + b
                    nc.vector.tensor_scalar(
                        out=tmp4[:, b, :],
                        in0=rb,
                        scalar1=r1_sb[:, col : col + 1],
                        scalar2=None,
                        op0=OP.add,
                    )
                nc.vector.tensor_mul(e4, tmp4, e4)
                g = mt // 4
                ml = (mt % 4) * 128
                for b in range(B):
                    vt = vtp.tile([128, C], bf16, tag="vt", name="vt")
                    nc.sync.dma_start(out=vt, in_=v_out[g, b, ml : ml + 128, :])
                    for ch in range(2):
                        nc.tensor.matmul(
                            fusion_ps[b][ch],
                            vt[:, ch * 128 : (ch + 1) * 128],
                            e4[:, b, :],
                            start=(mt == 0),
                            stop=(mt == MT - 1),
                        )

            # ---- residual: fusion_pad = g_b * fusion + x ----------------
            for b in range(B):
                for ch in range(2):
                    nc.vector.scalar_tensor_tensor(
                        out=fp_sb[b][ch][:, 1:9, 2:66],
                        in0=fusion_ps[b][ch].rearrange("p (r w) -> p r w", w=Wd),
                        scalar=g_bcast[:, b : b + 1],
                        in1=x_sb[b][ch][:, 2 : 2 + NL].rearrange(
                            "p (r w) -> p r w", w=Wd
                        ),
                        op0=OP.mult,
                        op1=OP.add,
                    )

        # =================================================================
        # phase E: ConvTranspose2d -> 18-row output slab (host-stitched)
        # tap-outer loop so the 4 batches reuse each weight tile; the
        # (py,px) component is interleaved into a [128, 9, 128] stage so
        # the output DMA writes 512B-contiguous runs.
        # =================================================================
        with (
            tc.tile_pool(name="cps", bufs=1, space="PSUM") as cps,
            tc.tile_pool(name="osb", bufs=1) as osb,
        ):
            NOUT = 9 * Wd  # 576 spatial outputs per (b, py, px)
            for py in range(2):
                ost = [
                    osb.tile([128, 9, 2 * Wd], f32, tag=f"ost{b}", name=f"ost{b}")
                    for b in range(B)
                ]
                for px in range(2):
                    pss = [
                        cps.tile([128, NOUT], f32, tag=f"cps{b}", name=f"cps{b}")
                        for b in range(B)
                    ]
                    taps = [
                        (ky, kx, k)
                        for ky in (py, py + 2)
                        for kx in (px, px + 2)
                        for k in range(2)
                    ]
                    for ti, (ky, kx, k) in enumerate(taps):
                        ro = (py + ky) // 2 - py
                        ww = (px + kx) // 2 - 1
                        for b in range(B):
                            fp = fp_sb[b][k]
                            nc.tensor.matmul(
                                pss[b][:, 0:512],
                                wco_sb[ky][kx][k],
                                fp[:, ro : ro + 8, 2 + ww : 66 + ww],
                                start=(ti == 0),
                                stop=(ti == len(taps) - 1),
                            )
                            nc.tensor.matmul(
                                pss[b][:, 512:NOUT],
                                wco_sb[ky][kx][k],
                                fp[:, ro + 8, 2 + ww : 66 + ww],
                                start=(ti == 0),
                                stop=(ti == len(taps) - 1),
                            )
                    for b in range(B):
                        ov = ost[b].rearrange("p j (w q) -> p j w q", q=2)[
                            :, :, :, px
                        ]
                        psv = pss[b].rearrange("p (j w) -> p j w", w=Wd)
                        # bias on j=1..8 only: slab rows 0,1 (j=0) are
                        # completed by the neighbor's (biased) rows 16,17;
                        # global row 0 is patched on the host.
                        nc.scalar.activation(
                            out=ov[:, 1:9, :],
                            in_=psv[:, 1:9, :],
                            func=AF.Identity,
                            bias=bco_sb,
                        )
                        nc.scalar.activation(
                            out=ov[:, 0:1, :],
                            in_=psv[:, 0:1, :],
                            func=AF.Copy,
                        )
                for b in range(B):
                    nc.sync.dma_start(
                        out=out_p[b].rearrange("c (j t) w -> c j t w", t=2)[
                            :, :, 1 - py, :
                        ],
                        in_=ost[b],
                    )

    nc.finalize()
    return nc


# ---------------------------------------------------------------------------
# host side
# ---------------------------------------------------------------------------
def _host_prep(x, wq, bq, wv, bv, w_adj1, b_adj1, w_adj2, b_adj2, gamma, w_co, b_co):
    import ml_dtypes

    bf16 = ml_dtypes.bfloat16
    x = np.asarray(x, np.float32).reshape(B, C, HW)
    xpad = np.zeros((B, C, HW + 4), np.float32)
    xpad[:, :, 2 : 2 + HW] = x
    xpad = xpad.astype(bf16)

    wqT = np.ascontiguousarray(np.asarray(wq, np.float32).T).astype(bf16)
    wvT = np.ascontiguousarray(np.asarray(wv, np.float32).T).astype(bf16)

    # grouped conv -> block-diagonal [3, 256, 32]
    w1 = np.zeros((3, C, CR), np.float32)
    wa1 = np.asarray(w_adj1, np.float32)  # [32, 8, 3]
    for g in range(CR):
        w1[:, g * 8 : (g + 1) * 8, g] = wa1[g].T  # [8,3] -> [3,8]
    w1 = w1.astype(bf16)

    # conv2 with output channels permuted to [query(32) | key(32)]
    wa2 = np.asarray(w_adj2, np.float32)  # [64, 32, 3]
    perm = np.concatenate([np.arange(0, 64, 2), np.arange(1, 64, 2)])
    w2 = np.ascontiguousarray(wa2[perm].transpose(2, 1, 0)).astype(bf16)  # [3,32,64]
    b2p = np.ascontiguousarray(np.asarray(b_adj2, np.float32)[perm].reshape(2 * CR, 1))

    # convT weights: flip, swap I/O -> [ky, kx, c_in, c_out]
    wt = np.flip(np.asarray(w_co, np.float32), (2, 3)).transpose(1, 0, 2, 3)
    wco = np.ascontiguousarray(wt.transpose(2, 3, 1, 0)).astype(bf16)  # [4,4,256,128]

    bvb = np.ascontiguousarray(
        np.broadcast_to(np.asarray(bv, np.float32), (128, C)).astype(bf16)
    )
    bq_ = np.ascontiguousarray(np.asarray(bq, np.float32).reshape(C, 1))
    b1_ = np.ascontiguousarray(np.asarray(b_adj1, np.float32).reshape(CR, 1))
    bco_ = np.ascontiguousarray(np.asarray(b_co, np.float32).reshape(C // 2, 1))
    gm = np.ascontiguousarray(np.asarray(gamma, np.float32).reshape(1, 1))

    in_maps = []
    for i in range(NCORES):
        n0 = i * NL
        xsl = np.ascontiguousarray(xpad[:, :, n0 : n0 + XW])
        j = np.arange(XW)
        valid = ((n0 - 2 + j) >= 0) & ((n0 - 2 + j) < HW)
        mask = np.ascontiguousarray(
            np.broadcast_to(valid.astype(np.float32), (128, XW)).astype(bf16)
        )
        in_maps.append(
            dict(
                xs=xsl,
                wqT=wqT,
                wvT=wvT,
                w1=w1,
                w2=w2,
                wco=wco,
                bq=bq_,
                b1=b1_,
                b2p=b2p,
                bco=bco_,
                bvb=bvb,
                mask=mask,
                gammas=gm,
            )
        )
    return in_maps


def _stitch(outs):
    full = np.zeros((B, C // 2, 2 * H, 2 * Wd), np.float32)
    for i in range(NCORES):
        y0 = 16 * i - 1
        lo = max(0, y0)
        hi = min(2 * H, y0 + OUTROWS)
        full[:, :, lo:hi, :] += outs[i][:, :, lo - y0 : hi - y0, :]
    return full


def _get_nc():
    if "nc" not in _CACHE:
        _CACHE["nc"] = build_module()
    return _CACHE["nc"]


def run_spmd(in_maps, trace=False, **kw):
    from concourse.bass_utils import run_bass_kernel_spmd

    nc = _get_nc()
    return run_bass_kernel_spmd(
        nc, in_maps, core_ids=list(range(NCORES)), trace=trace, **kw
    )


def kernel(x, wq, bq, wv, bv, w_adj1, b_adj1, w_adj2, b_adj2, gamma, w_co, b_co):
    in_maps = _host_prep(
        x, wq, bq, wv, bv, w_adj1, b_adj1, w_adj2, b_adj2, gamma, w_co, b_co
    )
    res = run_spmd(in_maps)
    full = _stitch([r["out"] for r in res.results])
    # slab rows 0,1 carry no bias (the neighbor's rows complete them);
    # global row 0 has no neighbor, so add the bias here.
    full[:, :, 0, :] += np.asarray(b_co, np.float32)[None, :, None]
    return full.astype(np.float32)
